# revision 3
# baseline (speedup 1.0000x reference)
"""Trainium2 Bass kernel for nn_AtomicNeuralNetwork — v4: int8 inputs.

Same compute structure as v3 (dense 5-atom interleave, full-K zero-padded
cross matmuls, a2-dup, split ps1/ps2 pools), but desc ships as INT8 with a
per-species scale folded into the W1 stationary blocks, halving input DMA
bytes (12.4MB -> 6.2MB/core).  DVE upcasts int8 -> bf16 per unit (2x perf
mode, ~0.66us/unit) before the L1 matmuls.  Input DMA is 8 transfers/iter
(supergroups of 13 units, 6.6KB descriptors), split sync:gpsimd 3:1 so a
slow-DMA device state still overlaps under compute; outputs ride the ACT
ring.  Measured rel err (CPU emulation): 1.21e-2 vs the 2e-2 gate.
"""

import sys

for _p in ("/opt/trn_rl_repo",):
    if _p not in sys.path:
        sys.path.append(_p)

import numpy as np
import ml_dtypes

import concourse.bass as bass  # noqa: F401
import concourse.mybir as mybir
import concourse.tile as tile
from concourse import bacc
from concourse import bass_utils

N, A, D, H, S = 4096, 256, 39, 50, 8
NCORES = 8
NF = N // NCORES            # frames per core
MM_DT = mybir.dt.bfloat16
NP_MM = ml_dtypes.bfloat16

UA = 5                      # atoms per unit
KD = 3 * D + 1              # moving rows per region: 3 descs + ones = 118
SGU = 13                    # units per input supergroup / output collector
OUT_U = 13
CAR = 8.0                   # bias carrier: bf16(silu(8.0)) == 8.0 exactly
WSEG = 6 * 128 + 16         # stationary image cols per species pattern = 784

LAST = {}
CAST_DMA = False   # gpsimd casts int8->bf16 during the input DMA


def _units(numbers):
    order = np.argsort(numbers, kind="stable").astype(np.int64)
    nunit = -(-A // UA)
    nunit = -(-nunit // SGU) * SGU               # multiple of SGU
    nslot = UA * nunit
    slots = np.concatenate([order, np.full(nslot - A, order[-1], np.int64)])
    valid = np.zeros(nslot, bool)
    valid[:A] = True
    sp5 = np.asarray(numbers)[slots].reshape(nunit, UA)
    patterns = {}
    unit_pat = []
    for u in range(nunit):
        key = tuple(int(x) for x in sp5[u])
        if key not in patterns:
            patterns[key] = len(patterns)
        unit_pat.append(patterns[key])
    return slots, valid, unit_pat, list(patterns.keys())


def _weight_images(pats, s_sp, W1, b1, W2, b2, W3, b3):
    """[128, WSEG*npat] f32 stationary image; W1 blocks pre-scaled by the
    per-species int8 scale (moving desc is the quantized integer code)."""
    npat = len(pats)
    img = np.zeros((128, WSEG * npat), np.float32)
    W1s = [W1[t] * s_sp[t] for t in range(S)]
    for p, (t0, t1, t2, t3, t4) in enumerate(pats):
        c = p * WSEG
        # S1A: K rows = [desc a2 | a0 | a1 | ones], M cols = z1A
        img[0:39, c + 100:c + 125] = W1s[t2][:, 0:25]
        img[39:78, c + 0:c + 50] = W1s[t0]
        img[78:117, c + 50:c + 100] = W1s[t1]
        img[117, c + 0:c + 50] = b1[t0]
        img[117, c + 50:c + 100] = b1[t1]
        img[117, c + 100:c + 125] = b1[t2][0:25]
        img[117, c + 125] = CAR
        # S1B: K rows = [desc a2 | a3 | a4 | ones], M = z1B
        c1 = c + 128
        img[0:39, c1 + 100:c1 + 125] = W1s[t2][:, 25:50]
        img[39:78, c1 + 0:c1 + 50] = W1s[t3]
        img[78:117, c1 + 50:c1 + 100] = W1s[t4]
        img[117, c1 + 0:c1 + 50] = b1[t3]
        img[117, c1 + 50:c1 + 100] = b1[t4]
        img[117, c1 + 100:c1 + 125] = b1[t2][25:50]
        img[117, c1 + 125] = CAR
        # S2A
        c2 = c + 256
        img[0:50, c2 + 0:c2 + 50] = W2[t0]
        img[50:100, c2 + 50:c2 + 100] = W2[t1]
        img[100:125, c2 + 100:c2 + 125] = W2[t2][0:25, 0:25]
        img[125, c2 + 0:c2 + 50] = b2[t0] / CAR
        img[125, c2 + 50:c2 + 100] = b2[t1] / CAR
        img[125, c2 + 100:c2 + 125] = b2[t2][0:25] / CAR
        img[125, c2 + 125] = 1.0
        # S2B
        c3 = c + 384
        img[0:50, c3 + 0:c3 + 50] = W2[t3]
        img[50:100, c3 + 50:c3 + 100] = W2[t4]
        img[100:125, c3 + 100:c3 + 125] = W2[t2][25:50, 25:50]
        img[125, c3 + 0:c3 + 50] = b2[t3] / CAR
        img[125, c3 + 50:c3 + 100] = b2[t4] / CAR
        img[125, c3 + 100:c3 + 125] = b2[t2][25:50] / CAR
        img[125, c3 + 125] = 1.0
        # S2XA / S2XB crosses (full-K, zero-padded)
        c4 = c + 512
        img[100:125, c4 + 100:c4 + 125] = W2[t2][25:50, 0:25]
        c5 = c + 640
        img[100:125, c5 + 100:c5 + 125] = W2[t2][0:25, 25:50]
        # S3A / S3B
        c6 = c + 768
        img[0:50, c6 + 0] = W3[t0][:, 0]
        img[50:100, c6 + 1] = W3[t1][:, 0]
        img[100:125, c6 + 2] = W3[t2][0:25, 0]
        img[125, c6 + 0:c6 + 5] = b3[[t0, t1, t2, t3, t4], 0] / CAR
        c7 = c + 776
        img[100:125, c7 + 2] = W3[t2][25:50, 0]
        img[0:50, c7 + 3] = W3[t3][:, 0]
        img[50:100, c7 + 4] = W3[t4][:, 0]
    return img


def _prepare(desc, numbers, W1, b1, W2, b2, W3, b3):
    desc = np.asarray(desc, np.float32)
    numbers = np.asarray(numbers).astype(np.int64)
    W1 = np.asarray(W1, np.float32); b1 = np.asarray(b1, np.float32)
    W2 = np.asarray(W2, np.float32); b2 = np.asarray(b2, np.float32)
    W3 = np.asarray(W3, np.float32); b3 = np.asarray(b3, np.float32)

    slots, valid, unit_pat, pats = _units(numbers)
    nunit = len(unit_pat)
    nsg = nunit // SGU

    # per-species int8 scales (same for every core: numbers are global)
    s_sp = np.zeros(S, np.float32)
    for sp in range(S):
        m = numbers == sp
        if m.any():
            s_sp[sp] = np.abs(desc[:, m, :]).max() / 127.0
    s_sp[s_sp == 0] = 1.0
    s_atom = s_sp[numbers]                                  # [A]
    q = np.clip(np.round(desc / s_atom[None, :, None]), -127, 127) \
        .astype(np.int8)                                    # [N, A, D]

    wimg = _weight_images(pats, s_sp, W1, b1, W2, b2, W3, b3).astype(NP_MM)

    sela = np.empty((nunit, 3), np.int64)
    selb = np.empty((nunit, 3), np.int64)
    for u in range(nunit):
        sela[u] = (5 * u + 2, 5 * u, 5 * u + 1)      # [a2 | a0 | a1]
        selb[u] = (5 * u + 2, 5 * u + 3, 5 * u + 4)  # [a2 | a3 | a4]

    in_maps = []
    for c in range(NCORES):
        at = q[c * NF:(c + 1) * NF][:, slots, :]             # [NF, nslot, D]
        at = np.ascontiguousarray(at.transpose(1, 2, 0))     # [nslot, D, NF]
        da = np.empty((nunit, KD, NF), np.int8)
        da[:, 0:3 * D] = at[sela.reshape(-1)].reshape(nunit, 3 * D, NF)
        da[:, 3 * D] = 1
        db = np.empty((nunit, KD, NF), np.int8)
        db[:, 0:3 * D] = at[selb.reshape(-1)].reshape(nunit, 3 * D, NF)
        db[:, 3 * D] = 1
        da = da.reshape(nsg, SGU, KD, NF)
        da = np.ascontiguousarray(da.transpose(0, 2, 1, 3)).reshape(nsg, KD, SGU * NF)
        db = db.reshape(nsg, SGU, KD, NF)
        db = np.ascontiguousarray(db.transpose(0, 2, 1, 3)).reshape(nsg, KD, SGU * NF)
        in_maps.append({"desc_a": da, "desc_b": db, "wt_in": wimg})

    meta = dict(unit_pat=unit_pat, npat=len(pats), nunit=nunit,
                slots=slots, valid=valid)
    return in_maps, meta


def _build(meta, repeat=0):
    import contextlib

    unit_pat = meta["unit_pat"]
    npat = meta["npat"]
    nunit = meta["nunit"]
    nsg = nunit // SGU
    SGW = SGU * NF                                  # supergroup cols = 6656

    nc = bacc.Bacc("TRN2", target_bir_lowering=False, debug=False)
    desc_a = nc.dram_tensor("desc_a", [nsg, KD, SGW], mybir.dt.int8,
                            kind="ExternalInput")
    desc_b = nc.dram_tensor("desc_b", [nsg, KD, SGW], mybir.dt.int8,
                            kind="ExternalInput")
    wt_in = nc.dram_tensor("wt_in", [128, WSEG * npat], MM_DT,
                           kind="ExternalInput")
    notile = -(-nunit // OUT_U)
    out = nc.dram_tensor("out", [notile, UA, OUT_U * NF], mybir.dt.float32,
                         kind="ExternalOutput")

    Silu = mybir.ActivationFunctionType.Silu
    F32 = mybir.dt.float32

    with tile.TileContext(nc) as tc:
        with (
            tc.tile_pool(name="w", bufs=1) as wpool,
            tc.tile_pool(name="dt8", bufs=2) as d8pool,
            tc.tile_pool(name="dtb", bufs=6) as dbpool,
            tc.tile_pool(name="h1", bufs=3) as h1pool,
            tc.tile_pool(name="h2", bufs=3) as h2pool,
            tc.tile_pool(name="o", bufs=2) as opool,
            tc.tile_pool(name="ps1", bufs=2, space="PSUM") as ps1pool,
            tc.tile_pool(name="ps2", bufs=2, space="PSUM") as ps2pool,
        ):
            wt = wpool.tile([128, WSEG * npat], MM_DT)
            nc.sync.dma_start(wt[:], wt_in[:])

            loop_cm = tc.For_i(0, repeat, 1) if repeat else contextlib.nullcontext()
            with loop_cm:
                sg_tiles = {}           # supergroup -> int8 tile (this body)
                dtb_tiles = {}          # unit -> upcast bf16 tile

                def emit_sg_dma(s):
                    if CAST_DMA:
                        tb = d8pool.tile([128, 2 * SGW], MM_DT, tag="dt8")
                        nc.gpsimd.dma_start(tb[0:KD, 0:SGW], desc_a[s, :, :])
                        nc.gpsimd.dma_start(tb[0:KD, SGW:2 * SGW],
                                            desc_b[s, :, :])
                        sg_tiles[s] = tb
                        return
                    t8 = d8pool.tile([128, 2 * SGW], mybir.dt.int8, tag="dt8")
                    qb = nc.gpsimd if s % 2 == 1 else nc.sync
                    nc.sync.dma_start(t8[0:KD, 0:SGW], desc_a[s, :, :])
                    qb.dma_start(t8[0:KD, SGW:2 * SGW], desc_b[s, :, :])
                    sg_tiles[s] = t8

                def emit_upcast(v):
                    s, j = divmod(v, SGU)
                    if s not in sg_tiles:
                        emit_sg_dma(s)
                    if CAST_DMA:
                        dtb_tiles[v] = (sg_tiles[s], j)
                        return
                    t8 = sg_tiles[s]
                    co = j * NF
                    dtb = dbpool.tile([128, 2 * NF], MM_DT, tag="dtb")
                    nc.vector.tensor_copy(dtb[0:KD, 0:NF], t8[0:KD, co:co + NF])
                    nc.vector.tensor_copy(dtb[0:KD, NF:2 * NF],
                                          t8[0:KD, SGW + co:SGW + co + NF])
                    dtb_tiles[v] = dtb

                UPC_LOOKAHEAD = 3

                def emit_l1(u):
                    if u + UPC_LOOKAHEAD < nunit:
                        emit_upcast(u + UPC_LOOKAHEAD)
                    ent = dtb_tiles.pop(u)
                    b = unit_pat[u] * WSEG
                    ps1 = ps1pool.tile([128, 2 * NF], F32, tag="ps1")
                    if CAST_DMA:
                        tb, j = ent
                        co = j * NF
                        mva = tb[0:KD, co:co + NF]
                        mvb = tb[0:KD, SGW + co:SGW + co + NF]
                    else:
                        mva = ent[0:KD, 0:NF]
                        mvb = ent[0:KD, NF:2 * NF]
                    nc.tensor.matmul(ps1[0:128, 0:NF], wt[0:KD, b:b + 128],
                                     mva, start=True, stop=True)
                    nc.tensor.matmul(ps1[0:128, NF:2 * NF],
                                     wt[0:KD, b + 128:b + 256],
                                     mvb, start=True, stop=True)
                    return ps1

                osb = None
                for v in range(UPC_LOOKAHEAD):
                    emit_upcast(v)
                ps1 = emit_l1(0)
                for u in range(nunit):
                    if u % OUT_U == 0:
                        osb = opool.tile([UA, OUT_U * NF], F32, tag="o")
                    b = unit_pat[u] * WSEG

                    h1 = h1pool.tile([128, 2 * NF], MM_DT, tag="h1")
                    nc.scalar.activation(h1[0:126, :], ps1[0:126, :], Silu)

                    ps2 = ps2pool.tile([128, 2 * NF], F32, tag="ps2")
                    nc.tensor.matmul(ps2[0:128, 0:NF], wt[0:126, b + 256:b + 384],
                                     h1[0:126, 0:NF], start=True, stop=False)
                    nc.tensor.matmul(ps2[0:128, 0:NF], wt[0:126, b + 512:b + 640],
                                     h1[0:126, NF:2 * NF], start=False, stop=True)
                    nc.tensor.matmul(ps2[0:128, NF:2 * NF],
                                     wt[0:126, b + 384:b + 512],
                                     h1[0:126, NF:2 * NF], start=True, stop=False)
                    nc.tensor.matmul(ps2[0:128, NF:2 * NF],
                                     wt[0:126, b + 640:b + 768],
                                     h1[0:126, 0:NF], start=False, stop=True)

                    if u + 1 < nunit:
                        ps1 = emit_l1(u + 1)

                    h2 = h2pool.tile([128, 2 * NF], MM_DT, tag="h2")
                    nc.scalar.activation(h2[0:126, :], ps2[0:126, :], Silu)

                    nc.tensor.matmul(ps2[0:5, 0:NF], wt[0:126, b + 768:b + 773],
                                     h2[0:126, 0:NF], start=True, stop=False)
                    nc.tensor.matmul(ps2[0:5, 0:NF], wt[0:126, b + 776:b + 781],
                                     h2[0:126, NF:2 * NF], start=False, stop=True)
                    co_o = (u % OUT_U) * NF
                    nc.vector.tensor_copy(osb[0:UA, co_o:co_o + NF],
                                          ps2[0:UA, 0:NF])
                    if u % OUT_U == OUT_U - 1 or u == nunit - 1:
                        t = u // OUT_U
                        nc.scalar.dma_start(
                            out[t, :, 0:co_o + NF], osb[0:UA, 0:co_o + NF])

    nc.compile()
    return nc


def kernel(desc, numbers, W1, b1, W2, b2, W3, b3):
    in_maps, meta = _prepare(desc, numbers, W1, b1, W2, b2, W3, b3)
    nc = _build(meta)

    last_err = None
    for _attempt in range(3):
        try:
            res = bass_utils.run_bass_kernel_spmd(
                nc, in_maps, core_ids=list(range(NCORES)))
            break
        except Exception as e:
            last_err = e
            import time
            time.sleep(20)
    else:
        raise last_err

    LAST.update(nc=nc, in_maps=in_maps, res=res, meta=meta)

    slots, valid = meta["slots"], meta["valid"]
    out = np.empty((N, A), np.float32)
    for c in range(NCORES):
        oc = res.results[c]["out"]
        oc = oc.reshape(-1, UA, OUT_U, NF)
        oc = oc.transpose(0, 2, 1, 3).reshape(-1, NF)
        nslot = len(slots)
        out[c * NF:(c + 1) * NF, slots[valid]] = oc[:nslot][valid].T
    return out


# revision 4
# speedup vs baseline: 1.0465x; 1.0465x over previous
"""Trainium2 Bass kernel for nn_AtomicNeuralNetwork — v4: int8 inputs.

Same compute structure as v3 (dense 5-atom interleave, full-K zero-padded
cross matmuls, a2-dup, split ps1/ps2 pools), but desc ships as INT8 with a
per-species scale folded into the W1 stationary blocks, halving input DMA
bytes (12.4MB -> 6.2MB/core).  DVE upcasts int8 -> bf16 per unit (2x perf
mode, ~0.66us/unit) before the L1 matmuls.  Input DMA rides fine-grained
4-unit transfer groups (2KB descriptors, 26 transfers/iter, sync+gpsimd)
— measured 6% faster than 13-unit supergroups: smaller transfers interleave
with compute's SBUF traffic instead of colliding in bursts.  Outputs ride
the ACT ring.  Measured rel err: 1.21e-2 vs the 2e-2 gate.
"""

import sys

for _p in ("/opt/trn_rl_repo",):
    if _p not in sys.path:
        sys.path.append(_p)

import numpy as np
import ml_dtypes

import concourse.bass as bass  # noqa: F401
import concourse.mybir as mybir
import concourse.tile as tile
from concourse import bacc
from concourse import bass_utils

N, A, D, H, S = 4096, 256, 39, 50, 8
NCORES = 8
NF = N // NCORES            # frames per core
MM_DT = mybir.dt.bfloat16
NP_MM = ml_dtypes.bfloat16

UA = 5                      # atoms per unit
KD = 3 * D + 1              # moving rows per region: 3 descs + ones = 118
SGU = 4                     # units per input transfer group
OUT_U = 13
CAR = 8.0                   # bias carrier: bf16(silu(8.0)) == 8.0 exactly
WSEG = 7 * 128 + 16         # stationary cols per pattern (S1X block at c+784)

LAST = {}
CAST_DMA = False   # gpsimd casts int8->bf16 during the input DMA
SLIVER = False     # timing diag: ship only 16 cols per supergroup
UPC_LA = 3         # upcast lookahead (units)
DTB_BUFS = 6
DEDUP_B = False    # ship desc_b without the a2 rows; rebuild via DVE copy
QB3 = False        # desc_b alternates gpsimd/scalar instead of gpsimd/sync
L2X30 = False      # L2 crosses as [30,30] tile_position (less SBUF weight traffic)
ASPLIT = False     # desc_a alternates sync/scalar HWDGE rings
DEDUP_X = False    # 79-row desc_b + zero-padded K=39 L1X matmul (no dup bytes)


def _units(numbers):
    order = np.argsort(numbers, kind="stable").astype(np.int64)
    nunit = -(-A // UA)
    nunit = -(-nunit // SGU) * SGU               # multiple of SGU
    nslot = UA * nunit
    slots = np.concatenate([order, np.full(nslot - A, order[-1], np.int64)])
    valid = np.zeros(nslot, bool)
    valid[:A] = True
    sp5 = np.asarray(numbers)[slots].reshape(nunit, UA)
    patterns = {}
    unit_pat = []
    for u in range(nunit):
        key = tuple(int(x) for x in sp5[u])
        if key not in patterns:
            patterns[key] = len(patterns)
        unit_pat.append(patterns[key])
    return slots, valid, unit_pat, list(patterns.keys())


def _weight_images(pats, s_sp, W1, b1, W2, b2, W3, b3):
    """[128, WSEG*npat] f32 stationary image; W1 blocks pre-scaled by the
    per-species int8 scale (moving desc is the quantized integer code)."""
    npat = len(pats)
    img = np.zeros((128, WSEG * npat), np.float32)
    W1s = [W1[t] * s_sp[t] for t in range(S)]
    for p, (t0, t1, t2, t3, t4) in enumerate(pats):
        c = p * WSEG
        # S1A: K rows = [desc a2 | a0 | a1 | ones], M cols = z1A
        img[0:39, c + 100:c + 125] = W1s[t2][:, 0:25]
        img[39:78, c + 0:c + 50] = W1s[t0]
        img[78:117, c + 50:c + 100] = W1s[t1]
        img[117, c + 0:c + 50] = b1[t0]
        img[117, c + 50:c + 100] = b1[t1]
        img[117, c + 100:c + 125] = b1[t2][0:25]
        img[117, c + 125] = CAR
        # S1B: dup mode: K = [a2 | a3 | a4 | ones(117)]; dedup mode:
        # K = [a3 | a4 | ones(78)] with a2's part via S1X below
        c1 = c + 128
        if DEDUP_X:
            img[0:39, c1 + 0:c1 + 50] = W1s[t3]
            img[39:78, c1 + 50:c1 + 100] = W1s[t4]
            img[78, c1 + 0:c1 + 50] = b1[t3]
            img[78, c1 + 50:c1 + 100] = b1[t4]
            img[78, c1 + 100:c1 + 125] = b1[t2][25:50]
            img[78, c1 + 125] = CAR
            # S1X: a2 desc (A block rows 0:39) -> z1B cols 100:125
            c8 = c + 784
            img[0:39, c8 + 100:c8 + 125] = W1s[t2][:, 25:50]
        else:
            img[0:39, c1 + 100:c1 + 125] = W1s[t2][:, 25:50]
            img[39:78, c1 + 0:c1 + 50] = W1s[t3]
            img[78:117, c1 + 50:c1 + 100] = W1s[t4]
            img[117, c1 + 0:c1 + 50] = b1[t3]
            img[117, c1 + 50:c1 + 100] = b1[t4]
            img[117, c1 + 100:c1 + 125] = b1[t2][25:50]
            img[117, c1 + 125] = CAR
        # S2A
        c2 = c + 256
        img[0:50, c2 + 0:c2 + 50] = W2[t0]
        img[50:100, c2 + 50:c2 + 100] = W2[t1]
        img[100:125, c2 + 100:c2 + 125] = W2[t2][0:25, 0:25]
        img[125, c2 + 0:c2 + 50] = b2[t0] / CAR
        img[125, c2 + 50:c2 + 100] = b2[t1] / CAR
        img[125, c2 + 100:c2 + 125] = b2[t2][0:25] / CAR
        img[125, c2 + 125] = 1.0
        # S2B
        c3 = c + 384
        img[0:50, c3 + 0:c3 + 50] = W2[t3]
        img[50:100, c3 + 50:c3 + 100] = W2[t4]
        img[100:125, c3 + 100:c3 + 125] = W2[t2][25:50, 25:50]
        img[125, c3 + 0:c3 + 50] = b2[t3] / CAR
        img[125, c3 + 50:c3 + 100] = b2[t4] / CAR
        img[125, c3 + 100:c3 + 125] = b2[t2][25:50] / CAR
        img[125, c3 + 125] = 1.0
        # S2XA / S2XB crosses (full-K, zero-padded)
        c4 = c + 512
        img[100:125, c4 + 100:c4 + 125] = W2[t2][25:50, 0:25]
        c5 = c + 640
        img[100:125, c5 + 100:c5 + 125] = W2[t2][0:25, 25:50]
        # S3A / S3B
        c6 = c + 768
        img[0:50, c6 + 0] = W3[t0][:, 0]
        img[50:100, c6 + 1] = W3[t1][:, 0]
        img[100:125, c6 + 2] = W3[t2][0:25, 0]
        img[125, c6 + 0:c6 + 5] = b3[[t0, t1, t2, t3, t4], 0] / CAR
        c7 = c + 776
        img[100:125, c7 + 2] = W3[t2][25:50, 0]
        img[0:50, c7 + 3] = W3[t3][:, 0]
        img[50:100, c7 + 4] = W3[t4][:, 0]
    return img


def _prepare(desc, numbers, W1, b1, W2, b2, W3, b3):
    desc = np.asarray(desc, np.float32)
    numbers = np.asarray(numbers).astype(np.int64)
    W1 = np.asarray(W1, np.float32); b1 = np.asarray(b1, np.float32)
    W2 = np.asarray(W2, np.float32); b2 = np.asarray(b2, np.float32)
    W3 = np.asarray(W3, np.float32); b3 = np.asarray(b3, np.float32)

    slots, valid, unit_pat, pats = _units(numbers)
    nunit = len(unit_pat)
    nsg = nunit // SGU

    # per-species int8 scales (same for every core: numbers are global)
    s_sp = np.zeros(S, np.float32)
    for sp in range(S):
        m = numbers == sp
        if m.any():
            s_sp[sp] = np.abs(desc[:, m, :]).max() / 127.0
    s_sp[s_sp == 0] = 1.0
    s_atom = s_sp[numbers]                                  # [A]
    q = np.clip(np.round(desc / s_atom[None, :, None]), -127, 127) \
        .astype(np.int8)                                    # [N, A, D]

    wimg = _weight_images(pats, s_sp, W1, b1, W2, b2, W3, b3).astype(NP_MM)

    sela = np.empty((nunit, 3), np.int64)
    selb = np.empty((nunit, 3), np.int64)
    for u in range(nunit):
        sela[u] = (5 * u + 2, 5 * u, 5 * u + 1)      # [a2 | a0 | a1]
        selb[u] = (5 * u + 2, 5 * u + 3, 5 * u + 4)  # [a2 | a3 | a4]

    in_maps = []
    for c in range(NCORES):
        at = q[c * NF:(c + 1) * NF][:, slots, :]             # [NF, nslot, D]
        at = np.ascontiguousarray(at.transpose(1, 2, 0))     # [nslot, D, NF]
        da = np.empty((nunit, KD, NF), np.int8)
        da[:, 0:3 * D] = at[sela.reshape(-1)].reshape(nunit, 3 * D, NF)
        da[:, 3 * D] = 1
        if DEDUP_B or DEDUP_X:
            db = np.empty((nunit, 2 * D + 1, NF), np.int8)
            db[:, 0:2 * D] = at[selb[:, 1:].reshape(-1)].reshape(nunit, 2 * D, NF)
            db[:, 2 * D] = 1
        else:
            db = np.empty((nunit, KD, NF), np.int8)
            db[:, 0:3 * D] = at[selb.reshape(-1)].reshape(nunit, 3 * D, NF)
            db[:, 3 * D] = 1
        da = da.reshape(nsg, SGU, KD, NF)
        da = np.ascontiguousarray(da.transpose(0, 2, 1, 3)).reshape(nsg, KD, SGU * NF)
        kb = db.shape[1]
        db = db.reshape(nsg, SGU, kb, NF)
        db = np.ascontiguousarray(db.transpose(0, 2, 1, 3)).reshape(nsg, kb, SGU * NF)
        in_maps.append({"desc_a": da, "desc_b": db, "wt_in": wimg})

    meta = dict(unit_pat=unit_pat, npat=len(pats), nunit=nunit,
                slots=slots, valid=valid)
    return in_maps, meta


def _build(meta, repeat=0):
    import contextlib

    unit_pat = meta["unit_pat"]
    npat = meta["npat"]
    nunit = meta["nunit"]
    nsg = nunit // SGU
    SGW = SGU * NF                                  # supergroup cols = 6656

    nc = bacc.Bacc("TRN2", target_bir_lowering=False, debug=False)
    desc_a = nc.dram_tensor("desc_a", [nsg, KD, SGW], mybir.dt.int8,
                            kind="ExternalInput")
    KB = 79 if (DEDUP_B or DEDUP_X) else KD
    desc_b = nc.dram_tensor("desc_b", [nsg, KB, SGW], mybir.dt.int8,
                            kind="ExternalInput")
    wt_in = nc.dram_tensor("wt_in", [128, WSEG * npat], MM_DT,
                           kind="ExternalInput")
    notile = -(-nunit // OUT_U)
    out = nc.dram_tensor("out", [notile, UA, OUT_U * NF], mybir.dt.float32,
                         kind="ExternalOutput")

    Silu = mybir.ActivationFunctionType.Silu
    F32 = mybir.dt.float32

    with tile.TileContext(nc) as tc:
        with (
            tc.tile_pool(name="w", bufs=1) as wpool,
            tc.tile_pool(name="dt8", bufs=2) as d8pool,
            tc.tile_pool(name="dtb", bufs=DTB_BUFS) as dbpool,
            tc.tile_pool(name="h1", bufs=3) as h1pool,
            tc.tile_pool(name="h2", bufs=3) as h2pool,
            tc.tile_pool(name="o", bufs=2) as opool,
            tc.tile_pool(name="ps1", bufs=2, space="PSUM") as ps1pool,
            tc.tile_pool(name="ps2", bufs=2, space="PSUM") as ps2pool,
        ):
            wt = wpool.tile([128, WSEG * npat], MM_DT)
            nc.sync.dma_start(wt[:], wt_in[:])

            loop_cm = tc.For_i(0, repeat, 1) if repeat else contextlib.nullcontext()
            with loop_cm:
                sg_tiles = {}           # supergroup -> int8 tile (this body)
                dtb_tiles = {}          # unit -> upcast bf16 tile

                def emit_sg_dma(s):
                    if CAST_DMA:
                        tb = d8pool.tile([128, 2 * SGW], MM_DT, tag="dt8")
                        nc.gpsimd.dma_start(tb[0:KD, 0:SGW], desc_a[s, :, :])
                        nc.gpsimd.dma_start(tb[0:KD, SGW:2 * SGW],
                                            desc_b[s, :, :])
                        sg_tiles[s] = tb
                        return
                    t8 = d8pool.tile([128, 2 * SGW], mybir.dt.int8, tag="dt8")
                    if QB3:
                        qb = nc.gpsimd if s % 2 == 1 else nc.scalar
                    else:
                        qb = nc.gpsimd if s % 2 == 1 else nc.sync
                    blo = 39 if DEDUP_B else 0
                    bhi = 79 if DEDUP_X else KD
                    qa = nc.scalar if (ASPLIT and s % 2 == 1) else nc.sync
                    if SLIVER:
                        qa.dma_start(t8[0:KD, 0:16], desc_a[s, :, 0:16])
                        qb.dma_start(t8[blo:bhi, SGW:SGW + 16],
                                     desc_b[s, :, 0:16])
                    else:
                        qa.dma_start(t8[0:KD, 0:SGW], desc_a[s, :, :])
                        qb.dma_start(t8[blo:bhi, SGW:2 * SGW], desc_b[s, :, :])
                    sg_tiles[s] = t8

                def emit_upcast(v):
                    s, j = divmod(v, SGU)
                    if s not in sg_tiles:
                        emit_sg_dma(s)
                    if CAST_DMA:
                        dtb_tiles[v] = (sg_tiles[s], j)
                        return
                    t8 = sg_tiles[s]
                    co = j * NF
                    dtb = dbpool.tile([128, 2 * NF], MM_DT, tag="dtb")
                    nc.vector.tensor_copy(dtb[0:KD, 0:NF], t8[0:KD, co:co + NF])
                    if DEDUP_X:
                        nc.vector.tensor_copy(dtb[0:79, NF:2 * NF],
                                              t8[0:79, SGW + co:SGW + co + NF])
                    elif DEDUP_B:
                        # B rows 32:118 from the shipped payload (32-aligned
                        # partition start; rows 32:39 are stale and get
                        # overwritten by the a2 copy below)
                        nc.vector.tensor_copy(dtb[32:KD, NF:2 * NF],
                                              t8[32:KD, SGW + co:SGW + co + NF])
                        nc.vector.tensor_copy(dtb[0:39, NF:2 * NF],
                                              dtb[0:39, 0:NF])
                    else:
                        nc.vector.tensor_copy(dtb[0:KD, NF:2 * NF],
                                              t8[0:KD, SGW + co:SGW + co + NF])
                    dtb_tiles[v] = dtb

                UPC_LOOKAHEAD = UPC_LA

                def emit_l1(u):
                    if u + UPC_LOOKAHEAD < nunit:
                        emit_upcast(u + UPC_LOOKAHEAD)
                    ent = dtb_tiles.pop(u)
                    b = unit_pat[u] * WSEG
                    ps1 = ps1pool.tile([128, 2 * NF], F32, tag="ps1")
                    if CAST_DMA:
                        tb, j = ent
                        co = j * NF
                        mva = tb[0:KD, co:co + NF]
                        mvb = tb[0:KD, SGW + co:SGW + co + NF]
                    else:
                        mva = ent[0:KD, 0:NF]
                        mvb = ent[0:KD, NF:2 * NF]
                    nc.tensor.matmul(ps1[0:128, 0:NF], wt[0:KD, b:b + 128],
                                     mva, start=True, stop=True)
                    if DEDUP_X:
                        nc.tensor.matmul(ps1[0:128, NF:2 * NF],
                                         wt[0:79, b + 128:b + 256],
                                         ent[0:79, NF:2 * NF],
                                         start=True, stop=False)
                        nc.tensor.matmul(ps1[0:128, NF:2 * NF],
                                         wt[0:39, b + 784:b + 912],
                                         ent[0:39, 0:NF],
                                         start=False, stop=True)
                    else:
                        nc.tensor.matmul(ps1[0:128, NF:2 * NF],
                                         wt[0:KD, b + 128:b + 256],
                                         mvb, start=True, stop=True)
                    return ps1

                osb = None
                for v in range(UPC_LOOKAHEAD):
                    emit_upcast(v)
                ps1 = emit_l1(0)
                for u in range(nunit):
                    if u % OUT_U == 0:
                        osb = opool.tile([UA, OUT_U * NF], F32, tag="o")
                    b = unit_pat[u] * WSEG

                    h1 = h1pool.tile([128, 2 * NF], MM_DT, tag="h1")
                    nc.scalar.activation(h1[0:126, :], ps1[0:126, :], Silu)

                    ps2 = ps2pool.tile([128, 2 * NF], F32, tag="ps2")
                    nc.tensor.matmul(ps2[0:128, 0:NF], wt[0:126, b + 256:b + 384],
                                     h1[0:126, 0:NF], start=True, stop=False)
                    if L2X30:
                        nc.tensor.matmul(ps2[96:126, 0:NF],
                                         wt[96:126, b + 512 + 96:b + 512 + 126],
                                         h1[96:126, NF:2 * NF],
                                         start=False, stop=True,
                                         tile_position=(96, 96))
                    else:
                        nc.tensor.matmul(ps2[0:128, 0:NF],
                                         wt[0:126, b + 512:b + 640],
                                         h1[0:126, NF:2 * NF],
                                         start=False, stop=True)
                    nc.tensor.matmul(ps2[0:128, NF:2 * NF],
                                     wt[0:126, b + 384:b + 512],
                                     h1[0:126, NF:2 * NF], start=True, stop=False)
                    if L2X30:
                        nc.tensor.matmul(ps2[96:126, NF:2 * NF],
                                         wt[96:126, b + 640 + 96:b + 640 + 126],
                                         h1[96:126, 0:NF],
                                         start=False, stop=True,
                                         tile_position=(96, 96))
                    else:
                        nc.tensor.matmul(ps2[0:128, NF:2 * NF],
                                         wt[0:126, b + 640:b + 768],
                                         h1[0:126, 0:NF], start=False, stop=True)

                    if u + 1 < nunit:
                        ps1 = emit_l1(u + 1)

                    h2 = h2pool.tile([128, 2 * NF], MM_DT, tag="h2")
                    nc.scalar.activation(h2[0:126, :], ps2[0:126, :], Silu)

                    nc.tensor.matmul(ps2[0:5, 0:NF], wt[0:126, b + 768:b + 773],
                                     h2[0:126, 0:NF], start=True, stop=False)
                    nc.tensor.matmul(ps2[0:5, 0:NF], wt[0:126, b + 776:b + 781],
                                     h2[0:126, NF:2 * NF], start=False, stop=True)
                    co_o = (u % OUT_U) * NF
                    nc.vector.tensor_copy(osb[0:UA, co_o:co_o + NF],
                                          ps2[0:UA, 0:NF])
                    if u % OUT_U == OUT_U - 1 or u == nunit - 1:
                        t = u // OUT_U
                        nc.scalar.dma_start(
                            out[t, :, 0:co_o + NF], osb[0:UA, 0:co_o + NF])

    nc.compile()
    return nc


def kernel(desc, numbers, W1, b1, W2, b2, W3, b3):
    in_maps, meta = _prepare(desc, numbers, W1, b1, W2, b2, W3, b3)
    nc = _build(meta)

    last_err = None
    for _attempt in range(3):
        try:
            res = bass_utils.run_bass_kernel_spmd(
                nc, in_maps, core_ids=list(range(NCORES)))
            break
        except Exception as e:
            last_err = e
            import time
            time.sleep(20)
    else:
        raise last_err

    LAST.update(nc=nc, in_maps=in_maps, res=res, meta=meta)

    slots, valid = meta["slots"], meta["valid"]
    out = np.empty((N, A), np.float32)
    for c in range(NCORES):
        oc = res.results[c]["out"]
        oc = oc.reshape(-1, UA, OUT_U, NF)
        oc = oc.transpose(0, 2, 1, 3).reshape(-1, NF)
        nslot = len(slots)
        out[c * NF:(c + 1) * NF, slots[valid]] = oc[:nslot][valid].T
    return out


# revision 5
# speedup vs baseline: 1.1246x; 1.0746x over previous
"""Trainium2 Bass kernel for nn_AtomicNeuralNetwork — v4: int8 inputs.

Same compute structure as v3 (dense 5-atom interleave, full-K zero-padded
cross matmuls, a2-dup, split ps1/ps2 pools), but desc ships as INT8 with a
per-species scale folded into the W1 stationary blocks, halving input DMA
bytes (12.4MB -> 6.2MB/core).  DVE upcasts int8 -> bf16 per unit (2x perf
mode, ~0.66us/unit) before the L1 matmuls.  Input DMA rides fine-grained
2-unit transfer groups (1KB descriptors, 52 transfers/iter, sync+gpsimd,
3-deep staging pool) — measured ~10%% faster than 13-unit supergroups:
small transfers interleave with compute's SBUF traffic instead of
colliding in bursts.  Outputs ride the ACT ring.  Measured rel err:
1.21e-2 vs the 2e-2 gate.
"""

import sys

for _p in ("/opt/trn_rl_repo",):
    if _p not in sys.path:
        sys.path.append(_p)

import numpy as np
import ml_dtypes

import concourse.bass as bass  # noqa: F401
import concourse.mybir as mybir
import concourse.tile as tile
from concourse import bacc
from concourse import bass_utils

N, A, D, H, S = 4096, 256, 39, 50, 8
NCORES = 8
NF = N // NCORES            # frames per core
MM_DT = mybir.dt.bfloat16
NP_MM = ml_dtypes.bfloat16

UA = 5                      # atoms per unit
KD = 3 * D + 1              # moving rows per region: 3 descs + ones = 118
SGU = 2                     # units per input transfer group
OUT_U = 13
CAR = 8.0                   # bias carrier: bf16(silu(8.0)) == 8.0 exactly
WSEG = 7 * 128 + 16         # stationary cols per pattern (S1X block at c+784)

LAST = {}
CAST_DMA = False   # gpsimd casts int8->bf16 during the input DMA
SLIVER = False     # timing diag: ship only 16 cols per supergroup
UPC_LA = 3         # upcast lookahead (units)
DTB_BUFS = 6
D8_BUFS = 3
DEDUP_B = False    # ship desc_b without the a2 rows; rebuild via DVE copy
QB3 = False        # desc_b alternates gpsimd/scalar instead of gpsimd/sync
L2X30 = False      # L2 crosses as [30,30] tile_position (less SBUF weight traffic)
ASPLIT = False     # desc_a alternates sync/scalar HWDGE rings
DEDUP_X = False    # 79-row desc_b + zero-padded K=39 L1X matmul (no dup bytes)


def _units(numbers):
    order = np.argsort(numbers, kind="stable").astype(np.int64)
    nunit = -(-A // UA)
    nunit = -(-nunit // SGU) * SGU               # multiple of SGU
    nslot = UA * nunit
    slots = np.concatenate([order, np.full(nslot - A, order[-1], np.int64)])
    valid = np.zeros(nslot, bool)
    valid[:A] = True
    sp5 = np.asarray(numbers)[slots].reshape(nunit, UA)
    patterns = {}
    unit_pat = []
    for u in range(nunit):
        key = tuple(int(x) for x in sp5[u])
        if key not in patterns:
            patterns[key] = len(patterns)
        unit_pat.append(patterns[key])
    return slots, valid, unit_pat, list(patterns.keys())


def _weight_images(pats, s_sp, W1, b1, W2, b2, W3, b3):
    """[128, WSEG*npat] f32 stationary image; W1 blocks pre-scaled by the
    per-species int8 scale (moving desc is the quantized integer code)."""
    npat = len(pats)
    img = np.zeros((128, WSEG * npat), np.float32)
    W1s = [W1[t] * s_sp[t] for t in range(S)]
    for p, (t0, t1, t2, t3, t4) in enumerate(pats):
        c = p * WSEG
        # S1A: K rows = [desc a2 | a0 | a1 | ones], M cols = z1A
        img[0:39, c + 100:c + 125] = W1s[t2][:, 0:25]
        img[39:78, c + 0:c + 50] = W1s[t0]
        img[78:117, c + 50:c + 100] = W1s[t1]
        img[117, c + 0:c + 50] = b1[t0]
        img[117, c + 50:c + 100] = b1[t1]
        img[117, c + 100:c + 125] = b1[t2][0:25]
        img[117, c + 125] = CAR
        # S1B: dup mode: K = [a2 | a3 | a4 | ones(117)]; dedup mode:
        # K = [a3 | a4 | ones(78)] with a2's part via S1X below
        c1 = c + 128
        if DEDUP_X:
            img[0:39, c1 + 0:c1 + 50] = W1s[t3]
            img[39:78, c1 + 50:c1 + 100] = W1s[t4]
            img[78, c1 + 0:c1 + 50] = b1[t3]
            img[78, c1 + 50:c1 + 100] = b1[t4]
            img[78, c1 + 100:c1 + 125] = b1[t2][25:50]
            img[78, c1 + 125] = CAR
            # S1X: a2 desc (A block rows 0:39) -> z1B cols 100:125
            c8 = c + 784
            img[0:39, c8 + 100:c8 + 125] = W1s[t2][:, 25:50]
        else:
            img[0:39, c1 + 100:c1 + 125] = W1s[t2][:, 25:50]
            img[39:78, c1 + 0:c1 + 50] = W1s[t3]
            img[78:117, c1 + 50:c1 + 100] = W1s[t4]
            img[117, c1 + 0:c1 + 50] = b1[t3]
            img[117, c1 + 50:c1 + 100] = b1[t4]
            img[117, c1 + 100:c1 + 125] = b1[t2][25:50]
            img[117, c1 + 125] = CAR
        # S2A
        c2 = c + 256
        img[0:50, c2 + 0:c2 + 50] = W2[t0]
        img[50:100, c2 + 50:c2 + 100] = W2[t1]
        img[100:125, c2 + 100:c2 + 125] = W2[t2][0:25, 0:25]
        img[125, c2 + 0:c2 + 50] = b2[t0] / CAR
        img[125, c2 + 50:c2 + 100] = b2[t1] / CAR
        img[125, c2 + 100:c2 + 125] = b2[t2][0:25] / CAR
        img[125, c2 + 125] = 1.0
        # S2B
        c3 = c + 384
        img[0:50, c3 + 0:c3 + 50] = W2[t3]
        img[50:100, c3 + 50:c3 + 100] = W2[t4]
        img[100:125, c3 + 100:c3 + 125] = W2[t2][25:50, 25:50]
        img[125, c3 + 0:c3 + 50] = b2[t3] / CAR
        img[125, c3 + 50:c3 + 100] = b2[t4] / CAR
        img[125, c3 + 100:c3 + 125] = b2[t2][25:50] / CAR
        img[125, c3 + 125] = 1.0
        # S2XA / S2XB crosses (full-K, zero-padded)
        c4 = c + 512
        img[100:125, c4 + 100:c4 + 125] = W2[t2][25:50, 0:25]
        c5 = c + 640
        img[100:125, c5 + 100:c5 + 125] = W2[t2][0:25, 25:50]
        # S3A / S3B
        c6 = c + 768
        img[0:50, c6 + 0] = W3[t0][:, 0]
        img[50:100, c6 + 1] = W3[t1][:, 0]
        img[100:125, c6 + 2] = W3[t2][0:25, 0]
        img[125, c6 + 0:c6 + 5] = b3[[t0, t1, t2, t3, t4], 0] / CAR
        c7 = c + 776
        img[100:125, c7 + 2] = W3[t2][25:50, 0]
        img[0:50, c7 + 3] = W3[t3][:, 0]
        img[50:100, c7 + 4] = W3[t4][:, 0]
    return img


def _prepare(desc, numbers, W1, b1, W2, b2, W3, b3):
    desc = np.asarray(desc, np.float32)
    numbers = np.asarray(numbers).astype(np.int64)
    W1 = np.asarray(W1, np.float32); b1 = np.asarray(b1, np.float32)
    W2 = np.asarray(W2, np.float32); b2 = np.asarray(b2, np.float32)
    W3 = np.asarray(W3, np.float32); b3 = np.asarray(b3, np.float32)

    slots, valid, unit_pat, pats = _units(numbers)
    nunit = len(unit_pat)
    nsg = nunit // SGU

    # per-species int8 scales (same for every core: numbers are global)
    s_sp = np.zeros(S, np.float32)
    for sp in range(S):
        m = numbers == sp
        if m.any():
            s_sp[sp] = np.abs(desc[:, m, :]).max() / 127.0
    s_sp[s_sp == 0] = 1.0
    s_atom = s_sp[numbers]                                  # [A]
    q = np.clip(np.round(desc / s_atom[None, :, None]), -127, 127) \
        .astype(np.int8)                                    # [N, A, D]

    wimg = _weight_images(pats, s_sp, W1, b1, W2, b2, W3, b3).astype(NP_MM)

    sela = np.empty((nunit, 3), np.int64)
    selb = np.empty((nunit, 3), np.int64)
    for u in range(nunit):
        sela[u] = (5 * u + 2, 5 * u, 5 * u + 1)      # [a2 | a0 | a1]
        selb[u] = (5 * u + 2, 5 * u + 3, 5 * u + 4)  # [a2 | a3 | a4]

    in_maps = []
    for c in range(NCORES):
        at = q[c * NF:(c + 1) * NF][:, slots, :]             # [NF, nslot, D]
        at = np.ascontiguousarray(at.transpose(1, 2, 0))     # [nslot, D, NF]
        da = np.empty((nunit, KD, NF), np.int8)
        da[:, 0:3 * D] = at[sela.reshape(-1)].reshape(nunit, 3 * D, NF)
        da[:, 3 * D] = 1
        if DEDUP_B or DEDUP_X:
            db = np.empty((nunit, 2 * D + 1, NF), np.int8)
            db[:, 0:2 * D] = at[selb[:, 1:].reshape(-1)].reshape(nunit, 2 * D, NF)
            db[:, 2 * D] = 1
        else:
            db = np.empty((nunit, KD, NF), np.int8)
            db[:, 0:3 * D] = at[selb.reshape(-1)].reshape(nunit, 3 * D, NF)
            db[:, 3 * D] = 1
        da = da.reshape(nsg, SGU, KD, NF)
        da = np.ascontiguousarray(da.transpose(0, 2, 1, 3)).reshape(nsg, KD, SGU * NF)
        kb = db.shape[1]
        db = db.reshape(nsg, SGU, kb, NF)
        db = np.ascontiguousarray(db.transpose(0, 2, 1, 3)).reshape(nsg, kb, SGU * NF)
        in_maps.append({"desc_a": da, "desc_b": db, "wt_in": wimg})

    meta = dict(unit_pat=unit_pat, npat=len(pats), nunit=nunit,
                slots=slots, valid=valid)
    return in_maps, meta


def _build(meta, repeat=0):
    import contextlib

    unit_pat = meta["unit_pat"]
    npat = meta["npat"]
    nunit = meta["nunit"]
    nsg = nunit // SGU
    SGW = SGU * NF                                  # supergroup cols = 6656

    nc = bacc.Bacc("TRN2", target_bir_lowering=False, debug=False)
    desc_a = nc.dram_tensor("desc_a", [nsg, KD, SGW], mybir.dt.int8,
                            kind="ExternalInput")
    KB = 79 if (DEDUP_B or DEDUP_X) else KD
    desc_b = nc.dram_tensor("desc_b", [nsg, KB, SGW], mybir.dt.int8,
                            kind="ExternalInput")
    wt_in = nc.dram_tensor("wt_in", [128, WSEG * npat], MM_DT,
                           kind="ExternalInput")
    notile = -(-nunit // OUT_U)
    out = nc.dram_tensor("out", [notile, UA, OUT_U * NF], mybir.dt.float32,
                         kind="ExternalOutput")

    Silu = mybir.ActivationFunctionType.Silu
    F32 = mybir.dt.float32

    with tile.TileContext(nc) as tc:
        with (
            tc.tile_pool(name="w", bufs=1) as wpool,
            tc.tile_pool(name="dt8", bufs=D8_BUFS) as d8pool,
            tc.tile_pool(name="dtb", bufs=DTB_BUFS) as dbpool,
            tc.tile_pool(name="h1", bufs=3) as h1pool,
            tc.tile_pool(name="h2", bufs=3) as h2pool,
            tc.tile_pool(name="o", bufs=2) as opool,
            tc.tile_pool(name="ps1", bufs=2, space="PSUM") as ps1pool,
            tc.tile_pool(name="ps2", bufs=2, space="PSUM") as ps2pool,
        ):
            wt = wpool.tile([128, WSEG * npat], MM_DT)
            nc.sync.dma_start(wt[:], wt_in[:])

            loop_cm = tc.For_i(0, repeat, 1) if repeat else contextlib.nullcontext()
            with loop_cm:
                sg_tiles = {}           # supergroup -> int8 tile (this body)
                dtb_tiles = {}          # unit -> upcast bf16 tile

                def emit_sg_dma(s):
                    if CAST_DMA:
                        tb = d8pool.tile([128, 2 * SGW], MM_DT, tag="dt8")
                        nc.gpsimd.dma_start(tb[0:KD, 0:SGW], desc_a[s, :, :])
                        nc.gpsimd.dma_start(tb[0:KD, SGW:2 * SGW],
                                            desc_b[s, :, :])
                        sg_tiles[s] = tb
                        return
                    t8 = d8pool.tile([128, 2 * SGW], mybir.dt.int8, tag="dt8")
                    if QB3:
                        qb = nc.gpsimd if s % 2 == 1 else nc.scalar
                    else:
                        qb = nc.gpsimd if s % 2 == 1 else nc.sync
                    blo = 39 if DEDUP_B else 0
                    bhi = 79 if DEDUP_X else KD
                    qa = nc.scalar if (ASPLIT and s % 2 == 1) else nc.sync
                    if SLIVER:
                        qa.dma_start(t8[0:KD, 0:16], desc_a[s, :, 0:16])
                        qb.dma_start(t8[blo:bhi, SGW:SGW + 16],
                                     desc_b[s, :, 0:16])
                    else:
                        qa.dma_start(t8[0:KD, 0:SGW], desc_a[s, :, :])
                        qb.dma_start(t8[blo:bhi, SGW:2 * SGW], desc_b[s, :, :])
                    sg_tiles[s] = t8

                def emit_upcast(v):
                    s, j = divmod(v, SGU)
                    if s not in sg_tiles:
                        emit_sg_dma(s)
                    if CAST_DMA:
                        dtb_tiles[v] = (sg_tiles[s], j)
                        return
                    t8 = sg_tiles[s]
                    co = j * NF
                    dtb = dbpool.tile([128, 2 * NF], MM_DT, tag="dtb")
                    nc.vector.tensor_copy(dtb[0:KD, 0:NF], t8[0:KD, co:co + NF])
                    if DEDUP_X:
                        nc.vector.tensor_copy(dtb[0:79, NF:2 * NF],
                                              t8[0:79, SGW + co:SGW + co + NF])
                    elif DEDUP_B:
                        # B rows 32:118 from the shipped payload (32-aligned
                        # partition start; rows 32:39 are stale and get
                        # overwritten by the a2 copy below)
                        nc.vector.tensor_copy(dtb[32:KD, NF:2 * NF],
                                              t8[32:KD, SGW + co:SGW + co + NF])
                        nc.vector.tensor_copy(dtb[0:39, NF:2 * NF],
                                              dtb[0:39, 0:NF])
                    else:
                        nc.vector.tensor_copy(dtb[0:KD, NF:2 * NF],
                                              t8[0:KD, SGW + co:SGW + co + NF])
                    dtb_tiles[v] = dtb

                UPC_LOOKAHEAD = UPC_LA

                def emit_l1(u):
                    if u + UPC_LOOKAHEAD < nunit:
                        emit_upcast(u + UPC_LOOKAHEAD)
                    ent = dtb_tiles.pop(u)
                    b = unit_pat[u] * WSEG
                    ps1 = ps1pool.tile([128, 2 * NF], F32, tag="ps1")
                    if CAST_DMA:
                        tb, j = ent
                        co = j * NF
                        mva = tb[0:KD, co:co + NF]
                        mvb = tb[0:KD, SGW + co:SGW + co + NF]
                    else:
                        mva = ent[0:KD, 0:NF]
                        mvb = ent[0:KD, NF:2 * NF]
                    nc.tensor.matmul(ps1[0:128, 0:NF], wt[0:KD, b:b + 128],
                                     mva, start=True, stop=True)
                    if DEDUP_X:
                        nc.tensor.matmul(ps1[0:128, NF:2 * NF],
                                         wt[0:79, b + 128:b + 256],
                                         ent[0:79, NF:2 * NF],
                                         start=True, stop=False)
                        nc.tensor.matmul(ps1[0:128, NF:2 * NF],
                                         wt[0:39, b + 784:b + 912],
                                         ent[0:39, 0:NF],
                                         start=False, stop=True)
                    else:
                        nc.tensor.matmul(ps1[0:128, NF:2 * NF],
                                         wt[0:KD, b + 128:b + 256],
                                         mvb, start=True, stop=True)
                    return ps1

                osb = None
                for v in range(UPC_LOOKAHEAD):
                    emit_upcast(v)
                ps1 = emit_l1(0)
                for u in range(nunit):
                    if u % OUT_U == 0:
                        osb = opool.tile([UA, OUT_U * NF], F32, tag="o")
                    b = unit_pat[u] * WSEG

                    h1 = h1pool.tile([128, 2 * NF], MM_DT, tag="h1")
                    nc.scalar.activation(h1[0:126, :], ps1[0:126, :], Silu)

                    ps2 = ps2pool.tile([128, 2 * NF], F32, tag="ps2")
                    nc.tensor.matmul(ps2[0:128, 0:NF], wt[0:126, b + 256:b + 384],
                                     h1[0:126, 0:NF], start=True, stop=False)
                    if L2X30:
                        nc.tensor.matmul(ps2[96:126, 0:NF],
                                         wt[96:126, b + 512 + 96:b + 512 + 126],
                                         h1[96:126, NF:2 * NF],
                                         start=False, stop=True,
                                         tile_position=(96, 96))
                    else:
                        nc.tensor.matmul(ps2[0:128, 0:NF],
                                         wt[0:126, b + 512:b + 640],
                                         h1[0:126, NF:2 * NF],
                                         start=False, stop=True)
                    nc.tensor.matmul(ps2[0:128, NF:2 * NF],
                                     wt[0:126, b + 384:b + 512],
                                     h1[0:126, NF:2 * NF], start=True, stop=False)
                    if L2X30:
                        nc.tensor.matmul(ps2[96:126, NF:2 * NF],
                                         wt[96:126, b + 640 + 96:b + 640 + 126],
                                         h1[96:126, 0:NF],
                                         start=False, stop=True,
                                         tile_position=(96, 96))
                    else:
                        nc.tensor.matmul(ps2[0:128, NF:2 * NF],
                                         wt[0:126, b + 640:b + 768],
                                         h1[0:126, 0:NF], start=False, stop=True)

                    if u + 1 < nunit:
                        ps1 = emit_l1(u + 1)

                    h2 = h2pool.tile([128, 2 * NF], MM_DT, tag="h2")
                    nc.scalar.activation(h2[0:126, :], ps2[0:126, :], Silu)

                    nc.tensor.matmul(ps2[0:5, 0:NF], wt[0:126, b + 768:b + 773],
                                     h2[0:126, 0:NF], start=True, stop=False)
                    nc.tensor.matmul(ps2[0:5, 0:NF], wt[0:126, b + 776:b + 781],
                                     h2[0:126, NF:2 * NF], start=False, stop=True)
                    co_o = (u % OUT_U) * NF
                    nc.vector.tensor_copy(osb[0:UA, co_o:co_o + NF],
                                          ps2[0:UA, 0:NF])
                    if u % OUT_U == OUT_U - 1 or u == nunit - 1:
                        t = u // OUT_U
                        nc.scalar.dma_start(
                            out[t, :, 0:co_o + NF], osb[0:UA, 0:co_o + NF])

    nc.compile()
    return nc


def kernel(desc, numbers, W1, b1, W2, b2, W3, b3):
    in_maps, meta = _prepare(desc, numbers, W1, b1, W2, b2, W3, b3)
    nc = _build(meta)

    last_err = None
    for _attempt in range(3):
        try:
            res = bass_utils.run_bass_kernel_spmd(
                nc, in_maps, core_ids=list(range(NCORES)))
            break
        except Exception as e:
            last_err = e
            import time
            time.sleep(20)
    else:
        raise last_err

    LAST.update(nc=nc, in_maps=in_maps, res=res, meta=meta)

    slots, valid = meta["slots"], meta["valid"]
    out = np.empty((N, A), np.float32)
    for c in range(NCORES):
        oc = res.results[c]["out"]
        oc = oc.reshape(-1, UA, OUT_U, NF)
        oc = oc.transpose(0, 2, 1, 3).reshape(-1, NF)
        nslot = len(slots)
        out[c * NF:(c + 1) * NF, slots[valid]] = oc[:nslot][valid].T
    return out
